# revision 1
# baseline (speedup 1.0000x reference)
"""DiffAttn Trainium2 kernel (8-core SPMD, no collectives).

Problem: B=2, T=2048, IN_DIM=OUT_DIM=1024, H=8 v-heads (2D=64), 2H=16 qk-heads (D=32).

Sharding: core c = 4*b + g handles batch b, head-group g: qk-heads {4g..4g+3}
(q/k cols 128g:128g+128 of E=512), v-heads {2g, 2g+1}, all T queries of its
batch. Out-projection is row-sharded: each core emits a partial out^T
[1024, 2048]; the host sums 4 partials per batch and transposes.

Layout is feature-major ("transposed") end to end: the host pre-transposes x,
q^T/k^T come from weights-stationary matmuls, dots are computed transposed
[keys, queries] so the exp'd scores feed attn@v directly as the moving
operand. v is produced token-major with a ones-column per v-head, so each
attn@v matmul (M=65) also emits the softmax denominator row for free.

Softmax skips the max-subtract (dots ~ N(0,1), exp can't overflow) and skips
the division entirely: the tail RMSNorm makes each output row scale-invariant,
so only the ratio r = lam*s1/s2 is needed, applied to the second qk-head's
attn@v output [64, 512] tile.

Bulk matmuls run in bf16 (fp32 PSUM accumulation; also keeps the PE HAM
clock-gate warm — fp32r matmuls measured 3x slower and permanently cold).
The softmax-denominator ratio path runs in f32/f32r. Per-query scalars
(r, 1/rms) are broadcast across 64 partitions with tiny K=8 basis matmuls;
the denominator rows are fanned onto 8 partitions via SBUF-to-SBUF DMA so
one DVE reciprocal instruction covers all 8 query-blocks in parallel lanes.
"""
import math

import numpy as np

H = 8
D = 32
LAMBDA_INIT = 0.8 - 0.6 * math.exp(-0.3)
B, T, IN_DIM, OUT_DIM = 2, 2048, 1024, 1024
E = 2 * H * D  # 512

N_CORES = 8
GROUPS = 4            # head groups (cores per batch)
QB = 512              # query block (matmul free dim)
NQB = T // QB         # 4
KT = 128              # key tile (partition dim)
NKT = T // KT         # 16
NIN = IN_DIM // 128   # 8

_compiled = None


def _build():
    import concourse.bass as bass  # noqa: F401
    import concourse.mybir as mybir
    from concourse import bacc
    from concourse.tile import TileContext

    f32 = mybir.dt.float32
    f32r = mybir.dt.float32r
    bf16 = mybir.dt.bfloat16
    AF = mybir.ActivationFunctionType
    MUL = mybir.AluOpType.mult

    nc = bacc.Bacc("TRN2", target_bir_lowering=False, num_devices=N_CORES)

    xT = nc.dram_tensor("xT", [IN_DIM, T], bf16, kind="ExternalInput")
    wq = nc.dram_tensor("wq", [IN_DIM, 128], bf16, kind="ExternalInput")
    wk = nc.dram_tensor("wk", [IN_DIM, 128], bf16, kind="ExternalInput")
    wv = nc.dram_tensor("wv", [IN_DIM, 128], bf16, kind="ExternalInput")
    wo0 = nc.dram_tensor("wo0", [64, OUT_DIM], bf16, kind="ExternalInput")
    wo1 = nc.dram_tensor("wo1", [64, OUT_DIM], bf16, kind="ExternalInput")
    lam = nc.dram_tensor("lam", [128, 1], f32, kind="ExternalInput")
    gam = nc.dram_tensor("gam", [64, 1], f32, kind="ExternalInput")
    onec = nc.dram_tensor("onec", [64, 1], bf16, kind="ExternalInput")
    bas8 = nc.dram_tensor("bas8", [8, 512], f32r, kind="ExternalInput")
    outT = nc.dram_tensor("outT", [OUT_DIM, T], f32, kind="ExternalOutput")

    with TileContext(nc) as tc:
        with tc.tile_pool(name="persist", bufs=1) as pp:
            # ---- persistent SBUF ----
            wq_sb = pp.tile([128, NIN, 128], bf16)
            wk_sb = pp.tile([128, NIN, 128], bf16)
            wv_sb = pp.tile([128, NIN, 128], bf16)
            wo0_sb = pp.tile([64, OUT_DIM], bf16)
            wo1_sb = pp.tile([64, OUT_DIM], bf16)
            qT_sb = pp.tile([128, T], bf16)          # 4 qk-heads x 32 rows
            kT_sb = pp.tile([128, T], bf16)
            v_sb = pp.tile([128, NKT, 130], bf16)    # [t, kt, (vA|1|vB|1)]
            # attn@v outputs staged per (vh, qb); a1 rows are overwritten by
            # the combined pre-norm value in phase C
            a1st = [pp.tile([64, T], f32, name=f"a1st{v}") for v in range(2)]
            a2st = [pp.tile([64, T], f32, name=f"a2st{v}") for v in range(2)]
            finl = [pp.tile([64, T], bf16, name=f"finl{v}") for v in range(2)]
            # softmax denominator rows at partition 64: [s1|s2, 2qb+vh, q]
            srow_sb = pp.tile([128, 2, 2 * NQB, QB], f32)
            # denominator rows fanned onto partitions 0-7 via DMA:
            # row i = (2qb+vh); free slot 0 = s2, slot 1 = s1
            s16 = pp.tile([8, 2, QB], f32)
            ssrow = pp.tile([1, 2 * NQB, QB], f32)
            ss8 = pp.tile([8, QB], f32)
            lam_sb = pp.tile([128, 1], f32)
            gam_sb = pp.tile([64, 1], f32)
            ones_c = pp.tile([64, 1], bf16)
            bas8_sb = pp.tile([8, 512], f32r)
            warm = pp.tile([128, 512], bf16)

            # ---- input DMAs ----
            nc.sync.dma_start(out=lam_sb[:, :], in_=lam[:, :])
            nc.sync.dma_start(out=gam_sb[:, :], in_=gam[:, :])
            nc.sync.dma_start(out=ones_c[:, :], in_=onec[:, :])
            nc.sync.dma_start(out=bas8_sb[:, :], in_=bas8[:, :])
            nc.sync.dma_start(out=wq_sb[:, :, :],
                              in_=wq.rearrange("(c p) m -> p c m", p=128))
            nc.sync.dma_start(out=wk_sb[:, :, :],
                              in_=wk.rearrange("(c p) m -> p c m", p=128))
            nc.sync.dma_start(out=wv_sb[:, :, :],
                              in_=wv.rearrange("(c p) m -> p c m", p=128))
            nc.sync.dma_start(out=wo0_sb[:, :], in_=wo0[:, :])
            nc.sync.dma_start(out=wo1_sb[:, :], in_=wo1[:, :])

            # ---- phase A: x^T load (per chunk), warmup, projections ----
            with (
                tc.tile_pool(name="xpool", bufs=1) as xp,
                tc.tile_pool(name="psA", bufs=2, space="PSUM") as psA,
            ):
                xT_sb = xp.tile([128, NIN, T], bf16)
                for c in range(NIN):
                    nc.sync.dma_start(out=xT_sb[:, c, :],
                                      in_=xT[128 * c:128 * (c + 1), :])

                nc.vector.memset(warm[:, :], 0.0)
                wm = psA.tile([128, 512], f32, tag="warm", bufs=1)
                for _ in range(24):
                    nc.tensor.matmul(wm[:, :], warm[:, :128], warm[:, :],
                                     start=True, stop=True)
                nc.scalar.activation(warm[:64, :], wm[:64, :], AF.Exp)

                for dst, w_sb in ((qT_sb, wq_sb), (kT_sb, wk_sb)):
                    for tb in range(NQB):
                        p = psA.tile([128, QB], f32, tag="proj")
                        for c in range(NIN):
                            nc.tensor.matmul(
                                p[:, :], w_sb[:, c, :],
                                xT_sb[:, c, tb * QB:(tb + 1) * QB],
                                start=(c == 0), stop=(c == NIN - 1))
                        nc.vector.tensor_copy(dst[:, tb * QB:(tb + 1) * QB],
                                              p[:, :])
                for kt in range(NKT):
                    p = psA.tile([128, 128], f32, tag="vproj")
                    for c in range(NIN):
                        nc.tensor.matmul(
                            p[:, :], xT_sb[:, c, kt * 128:(kt + 1) * 128],
                            wv_sb[:, c, :], start=(c == 0), stop=(c == NIN - 1))
                    nc.vector.tensor_copy(v_sb[:, kt, 0:64], p[:, 0:64])
                    nc.vector.tensor_copy(v_sb[:, kt, 65:129], p[:, 64:128])
                with tc.tile_pool(name="onescr", bufs=1) as op_:
                    oscr = op_.tile([128, NKT], f32)
                    nc.vector.memset(oscr[:, :], 1.0)
                    nc.vector.tensor_copy(
                        v_sb[:, :, 64:65].rearrange("p n 1 -> p n"),
                        oscr[:, :])
                    nc.vector.tensor_copy(
                        v_sb[:, :, 129:130].rearrange("p n 1 -> p n"),
                        oscr[:, :])

            # ---- phase B: attention ----
            with (
                tc.tile_pool(name="dots_ps", bufs=1, space="PSUM") as dps,
                tc.tile_pool(name="acc_ps", bufs=1, space="PSUM") as aps,
                tc.tile_pool(name="epool", bufs=3) as ep,
            ):
                for qb in range(NQB):
                    qs = qb * QB
                    accs = [aps.tile([65, QB], f32, tag=f"acc{j}",
                                     name=f"acc{j}_{qb}") for j in range(4)]
                    es = {}
                    for kt in range(NKT):
                        # all four dots matmuls issue back-to-back into
                        # distinct 32-row PE tiles -> fully concurrent
                        dps_t = [dps.tile([128, 2 * QB], f32, tag=f"d{half}",
                                          name=f"d{half}_{qb}_{kt}")
                                 for half in range(2)]
                        for h in range(4):
                            dp = dps_t[h // 2]
                            nc.tensor.matmul(
                                dp[:, (h % 2) * QB:(h % 2 + 1) * QB],
                                kT_sb[32 * h:32 * (h + 1),
                                      kt * KT:(kt + 1) * KT],
                                qT_sb[32 * h:32 * (h + 1), qs:qs + QB],
                                start=True, stop=True,
                                tile_position=(32 * h, 0))
                        for half in range(2):
                            e = ep.tile([128, 2 * QB], bf16, tag=f"e{half}",
                                        name=f"e{half}_{qb}_{kt}")
                            nc.scalar.activation(e[:, :], dps_t[half][:, :],
                                                 AF.Exp)
                            es[(kt, half)] = e
                        if kt > 0:
                            _attnv(nc, accs, es, v_sb, kt - 1, NKT)
                    _attnv(nc, accs, es, v_sb, NKT - 1, NKT)
                    # stage accumulators to SBUF (partition-aligned copies),
                    # then fan denominator rows onto partitions 0-7 via DMA
                    for vh in range(2):
                        a1, a2 = accs[2 * vh], accs[2 * vh + 1]
                        i = 2 * qb + vh
                        nc.vector.tensor_copy(a1st[vh][:, qs:qs + QB],
                                              a1[0:64, :])
                        nc.vector.tensor_copy(a2st[vh][:, qs:qs + QB],
                                              a2[0:64, :])
                        nc.vector.tensor_copy(srow_sb[64:65, 0, i, :],
                                              a1[64:65, :])
                        nc.vector.tensor_copy(srow_sb[64:65, 1, i, :],
                                              a2[64:65, :])
                        nc.sync.dma_start(out=s16[i:i + 1, 0, :],
                                          in_=srow_sb[64:65, 1, i, :])
                        nc.sync.dma_start(out=s16[i:i + 1, 1, :],
                                          in_=srow_sb[64:65, 0, i, :])

            # ---- phase C: combine, RMS norm, out-projection ----
            with (
                tc.tile_pool(name="psC", bufs=2, space="PSUM") as psC,
                tc.tile_pool(name="sbC", bufs=2) as sbC,
            ):
                # r' rows for all 8 (qb, vh) at once: r = (s1 * lam) / s2
                rec2 = sbC.tile([8, QB], f32, tag="rec2", bufs=1)
                nc.vector.reciprocal(rec2[:, :], s16[:, 0, :])
                r8 = sbC.tile([8, QB], f32, tag="r8", bufs=1)
                nc.vector.scalar_tensor_tensor(
                    r8[:, :], s16[:, 1, :], lam_sb[0:8, 0:1], rec2[:, :],
                    op0=MUL, op1=MUL)
                r8R = sbC.tile([8, QB], f32r, tag="r8R", bufs=1)
                nc.vector.tensor_copy(r8R[:, :], r8[:, :])
                # comb (in place over a1st) = a1 - r * a2
                for qb in range(NQB):
                    qs = qb * QB
                    for vh in range(2):
                        i = 2 * qb + vh
                        rb = psC.tile([64, QB], f32, tag="rb")
                        nc.tensor.matmul(rb[:, :],
                                         bas8_sb[:, 64 * i:64 * (i + 1)],
                                         r8R[:, :], start=True, stop=True)
                        t2 = sbC.tile([64, QB], f32, tag="t2")
                        nc.vector.tensor_mul(t2[:, :],
                                             a2st[vh][:, qs:qs + QB], rb[:, :])
                        nc.vector.tensor_sub(a1st[vh][:, qs:qs + QB],
                                             a1st[vh][:, qs:qs + QB], t2[:, :])
                # sumsq rows, fanned onto partitions 0-7 like the s rows
                for qb in range(NQB):
                    qs = qb * QB
                    for vh in range(2):
                        i = 2 * qb + vh
                        sq = sbC.tile([64, QB], bf16, tag="sq")
                        nc.scalar.activation(sq[:, :], a1st[vh][:, qs:qs + QB],
                                             AF.Square)
                        ss = psC.tile([1, QB], f32, tag="ss")
                        nc.tensor.matmul(ss[:, :], ones_c[:, :], sq[:, :],
                                         start=True, stop=True)
                        nc.vector.tensor_copy(ssrow[0:1, i, :], ss[:, :])
                        nc.sync.dma_start(out=ss8[i:i + 1, :],
                                          in_=ssrow[0:1, i, :])
                # rinv = 1/sqrt(sumsq/64) for all 8 rows in one go
                rn8 = sbC.tile([8, QB], f32, tag="rn8", bufs=1)
                nc.scalar.activation(rn8[:, :], ss8[:, :], AF.Sqrt,
                                     scale=1.0 / 64.0)
                nc.vector.reciprocal(rn8[:, :], rn8[:, :])
                rn8R = sbC.tile([8, QB], f32r, tag="rn8R", bufs=1)
                nc.vector.tensor_copy(rn8R[:, :], rn8[:, :])
                # final = comb * rinv_bcast * gam; out-projection per qb
                for qb in range(NQB):
                    qs = qb * QB
                    for vh in range(2):
                        i = 2 * qb + vh
                        rb = psC.tile([64, QB], f32, tag="rb")
                        nc.tensor.matmul(rb[:, :],
                                         bas8_sb[:, 64 * i:64 * (i + 1)],
                                         rn8R[:, :], start=True, stop=True)
                        nc.vector.scalar_tensor_tensor(
                            finl[vh][:, qs:qs + QB], a1st[vh][:, qs:qs + QB],
                            gam_sb[:, 0:1], rb[:, :], op0=MUL, op1=MUL)
                    for oc in range(OUT_DIM // 128):
                        p = psC.tile([128, QB], f32, tag="oproj", bufs=3)
                        nc.tensor.matmul(p[:, :],
                                         wo0_sb[:, oc * 128:(oc + 1) * 128],
                                         finl[0][:, qs:qs + QB],
                                         start=True, stop=False)
                        nc.tensor.matmul(p[:, :],
                                         wo1_sb[:, oc * 128:(oc + 1) * 128],
                                         finl[1][:, qs:qs + QB],
                                         start=False, stop=True)
                        o = sbC.tile([128, QB], f32, tag="ostage", bufs=3)
                        nc.vector.tensor_copy(o[:, :], p[:, :])
                        nc.sync.dma_start(
                            out=outT[oc * 128:(oc + 1) * 128, qs:qs + QB],
                            in_=o[:, :])

    nc.compile()
    return nc


def _attnv(nc, accs, es, v_sb, kt, nkt):
    for j in range(4):
        e = es[(kt, j // 2)]
        ecol = (j % 2) * QB
        vcol = 65 * (j // 2)
        nc.tensor.matmul(
            accs[j][:, :], v_sb[:, kt, vcol:vcol + 65],
            e[:, ecol:ecol + QB],
            start=(kt == 0), stop=(kt == nkt - 1))


def _get_compiled():
    global _compiled
    if _compiled is None:
        _compiled = _build()
    return _compiled


def make_in_maps(x, Wq, Wkv, Wout, lambda_q1, lambda_k1, lambda_q2, lambda_k2,
                 gamma):
    import ml_dtypes
    bf = ml_dtypes.bfloat16
    x = np.asarray(x, dtype=np.float32)
    Wq = np.asarray(Wq, dtype=np.float32)
    Wkv = np.asarray(Wkv, dtype=np.float32)
    Wout = np.asarray(Wout, dtype=np.float32)
    lam_v = (math.exp(float(np.dot(lambda_q1, lambda_k1)))
             - math.exp(float(np.dot(lambda_q2, lambda_k2))) + LAMBDA_INIT)
    lam_arr = np.full((128, 1), lam_v, dtype=np.float32)
    gam_arr = (np.asarray(gamma, dtype=np.float32)
               * (1.0 - LAMBDA_INIT)).reshape(64, 1).copy()
    onec = np.ones((64, 1), dtype=bf)
    bas8 = np.zeros((8, 512), dtype=np.float32)
    for i in range(8):
        bas8[i, 64 * i:64 * (i + 1)] = 1.0
    Wq_s = (Wq * (D ** -0.5)).astype(np.float32)
    Wk = Wkv[:, :E]
    Wv = Wkv[:, E:]
    xT_all = [np.ascontiguousarray(x[b].T).astype(bf) for b in range(B)]
    in_maps = []
    for c in range(N_CORES):
        b, g = divmod(c, GROUPS)
        sl = slice(128 * g, 128 * (g + 1))
        in_maps.append({
            "xT": xT_all[b],
            "wq": np.ascontiguousarray(Wq_s[:, sl]).astype(bf),
            "wk": np.ascontiguousarray(Wk[:, sl]).astype(bf),
            "wv": np.ascontiguousarray(Wv[:, sl]).astype(bf),
            "wo0": np.ascontiguousarray(Wout[128 * g:128 * g + 64, :]).astype(bf),
            "wo1": np.ascontiguousarray(Wout[128 * g + 64:128 * (g + 1), :]).astype(bf),
            "lam": lam_arr,
            "gam": gam_arr,
            "onec": onec,
            "bas8": bas8,
        })
    return in_maps


def kernel(x, Wq, Wkv, Wout, lambda_q1, lambda_k1, lambda_q2, lambda_k2,
           gamma, _run_kw=None):
    import sys
    if "/opt/trn_rl_repo" not in sys.path:
        sys.path.insert(0, "/opt/trn_rl_repo")
    from concourse.bass_utils import run_bass_kernel_spmd

    nc = _get_compiled()
    in_maps = make_in_maps(x, Wq, Wkv, Wout, lambda_q1, lambda_k1,
                           lambda_q2, lambda_k2, gamma)
    res = run_bass_kernel_spmd(nc, in_maps, list(range(N_CORES)),
                               **(_run_kw or {}))
    out = np.zeros((B, T, OUT_DIM), dtype=np.float32)
    for c in range(N_CORES):
        out[c // GROUPS] += res.results[c]["outT"].T
    kernel.last_result = res
    return out

